# revision 1
# baseline (speedup 1.0000x reference)
"""GPT2 self-attention on 8 trn2 NeuronCores (tensor-parallel).

Sharding (per the sharding hint): core c in 0..7 handles batch b = c//4 and
head-group g = c%4 (4 of 16 heads = 256 of 1024 dims).

Per core:
  1. QK^T projection:  [512 qk-dims, 2048 tokens] = w_qk^T @ x   (x^T as rhs)
  2. V   projection:   [2048 tokens, 256 v-dims]  = x @ w_v      (x^T as lhsT)
  3. Causal attention per head, keys on PSUM partitions:
       S^T = (K^T-tile).T @ Q^T  -> diag mask -> exp(S/8) on ACT -> probs bf16
       O^T_aug = [V | 1]^T @ probs   (row 64 = softmax denominators)
       normalize via reciprocal + DRAM-bounce partition-broadcast multiply
  4. AllGather(group of 4) of O^T [256, 2048] bf16 -> O^T_full [1024, 2048]
  5. Output projection, column-sharded: z[:, 256g:256g+256] for all 2048
     tokens with a host-sliced w_out column shard -> z [2048, 256]

Host only reorders/slices/casts inputs (x^T, weight slices, bf16) and places
the 8 per-core z column-chunks into [B, S, D]. b_qkv/b_out are zeros by the
problem spec (fill: zeros) and are folded out. Matmuls run bf16 with fp32
PSUM accumulation.
"""

import numpy as np
import ml_dtypes
from contextlib import ExitStack

B, S, D, H = 2, 2048, 1024, 16
HD = 64            # head dim
NCORES = 8
HPC = 4            # heads per core
GD = HPC * HD      # 256 dims per core group
QW = 512            # query-chunk width (1 PSUM bank)
NEG = -1.0e9

_CACHE = {}


def _build_program():
    import concourse.tile as tile
    from concourse import bacc, mybir

    bf16 = mybir.dt.bfloat16
    f32 = mybir.dt.float32

    nc = bacc.Bacc("TRN2", target_bir_lowering=False, debug=False,
                   num_devices=NCORES)

    xt = nc.dram_tensor("xt", [D, S], bf16, kind="ExternalInput").ap()
    wqk = nc.dram_tensor("wqk", [D, 2 * GD], bf16, kind="ExternalInput").ap()
    wv = nc.dram_tensor("wv", [D, GD], bf16, kind="ExternalInput").ap()
    wout = nc.dram_tensor("wout", [D, GD], bf16, kind="ExternalInput").ap()
    mneg = nc.dram_tensor("mneg", [128, 128], bf16, kind="ExternalInput").ap()
    mtri = nc.dram_tensor("mtri", [128, 128], bf16, kind="ExternalInput").ap()
    z_out = nc.dram_tensor("z", [S, GD], f32, kind="ExternalOutput").ap()

    NKT = S // 128          # 16 key tiles
    KD = D // 128           # 8 contraction tiles over d_model
    NQC = S // QW           # query chunks per head
    HS = S // 2             # token half width (for split gathers)

    with tile.TileContext(nc) as tc, ExitStack() as ctx:
        persist = ctx.enter_context(tc.tile_pool(name="persist", bufs=1))
        # flat PSUM budget: p1(2) + aps(4) + otps(2) = 8 banks
        p1ps = ctx.enter_context(tc.tile_pool(name="p1ps", bufs=2, space="PSUM"))
        aps = ctx.enter_context(tc.tile_pool(name="aps", bufs=4, space="PSUM"))
        otps = ctx.enter_context(tc.tile_pool(name="otps", bufs=2, space="PSUM"))
        probs_pool = ctx.enter_context(tc.tile_pool(name="probs_pool", bufs=4))
        dram_pool = ctx.enter_context(tc.tile_pool(name="dram_pool", bufs=1, space="DRAM"))
        z_pool = ctx.enter_context(tc.tile_pool(name="z_pool", bufs=3))

        xt_sb = [persist.tile([128, S], bf16, tag=f"xt{k}", name=f"xt{k}") for k in range(KD)]
        wqk_sb = [persist.tile([128, 2 * GD], bf16, tag=f"wqk{k}", name=f"wqk{k}") for k in range(KD)]
        wv_sb = [persist.tile([128, GD], bf16, tag=f"wv{k}", name=f"wv{k}") for k in range(KD)]
        mneg_sb = persist.tile([128, 128], bf16, tag="mneg", name="mneg_sb")
        mtri_sb = persist.tile([128, 128], bf16, tag="mtri", name="mtri_sb")
        qkt_sb = [persist.tile([128, S], bf16, tag=f"qkt{m}", name=f"qkt{m}") for m in range(4)]
        v_sb = [persist.tile([128, HPC, HD + 1], bf16, tag=f"v{t}", name=f"v{t}") for t in range(NKT)]
        ot_sb = [persist.tile([128, S], bf16, tag=f"ot{p}", name=f"ot{p}") for p in range(2)]
        otu_sb = [persist.tile([128, S], f32, tag=f"otu{p}", name=f"otu{p}") for p in range(2)]
        rec_sb = [persist.tile([64, S], f32, tag=f"rec{p}", name=f"rec{p}") for p in range(2)]
        bc_sb = [persist.tile([128, S], f32, tag=f"bc{p}", name=f"bc{p}") for p in range(2)]
        wout_sb = [persist.tile([128, GD], bf16, tag=f"wout{k}", name=f"wout{k}") for k in range(KD)]
        otf_sb = [persist.tile([128, S], bf16, tag=f"otf{k}", name=f"otf{k}") for k in range(KD)]
        zev_sb = [persist.tile([128, GD], f32, tag=f"zev{mt}", name=f"zev{mt}")
                  for mt in range(S // 128)]

        # spread initial loads across engine DMA queues
        nc.gpsimd.dma_start(out=mneg_sb[:], in_=mneg[:])
        nc.gpsimd.dma_start(out=mtri_sb[:], in_=mtri[:])
        for k in range(KD):
            e1 = nc.sync if k % 2 == 0 else nc.scalar
            e2 = nc.scalar if k % 2 == 0 else nc.sync
            e1.dma_start(out=xt_sb[k][:], in_=xt[k * 128:(k + 1) * 128, :])
            e2.dma_start(out=wqk_sb[k][:], in_=wqk[k * 128:(k + 1) * 128, :])
        for k in range(KD):
            nc.gpsimd.dma_start(out=wv_sb[k][:], in_=wv[k * 128:(k + 1) * 128, :])
        for k in range(KD):
            nc.gpsimd.dma_start(out=wout_sb[k][:], in_=wout[k * 128:(k + 1) * 128, :])

        def qkt_chunk(m, n):
            ps = p1ps.tile([128, 512], f32, tag="p1", name="p1ps_t")
            for k in range(KD):
                nc.tensor.matmul(
                    ps[:],
                    wqk_sb[k][:, m * 128:(m + 1) * 128],
                    xt_sb[k][:, n * 512:(n + 1) * 512],
                    start=(k == 0), stop=(k == KD - 1),
                )
            nc.vector.tensor_copy(qkt_sb[m][:, n * 512:(n + 1) * 512], ps[:])

        def v_tile(t):
            ps = p1ps.tile([128, GD], f32, tag="p1", name="p1vps_t")
            for k in range(KD):
                nc.tensor.matmul(
                    ps[:, 0:GD],
                    xt_sb[k][:, t * 128:(t + 1) * 128],
                    wv_sb[k][:],
                    start=(k == 0), stop=(k == KD - 1),
                )
            nc.vector.tensor_copy(
                v_sb[t][:, :, 0:HD],
                ps[:, 0:GD].rearrange("p (h d) -> p h d", h=HPC),
            )
            nc.vector.memset(v_sb[t][:, :, HD:HD + 1], 1.0)

        def attn_qc(pair, qc):
            qstart = qc * QW
            nkt = (qstart + QW) // 128
            otp = [otps.tile([HD + 1, QW], f32, tag="ot", name="otp_t")
                   for _ in range(2)]
            for kt in range(nkt):
                j = kt - qc * (QW // 128)
                qoff = max(0, 128 * j)
                pr = [None, None]
                for hh in range(2):
                    base = 64 * hh
                    sp = aps.tile([128, QW], f32, tag="sc", name="sc_t")
                    nc.tensor.matmul(
                        sp[:, qoff:QW],
                        qkt_sb[2 + pair][base:base + 64, kt * 128:(kt + 1) * 128],
                        qkt_sb[pair][base:base + 64,
                                     qstart + qoff:qstart + QW],
                        start=True, stop=(j < 0),
                    )
                    if j >= 0:
                        nc.tensor.matmul(
                            sp[:, qoff:qoff + 128],
                            mneg_sb[:],
                            mtri_sb[:],
                            start=False, stop=True,
                        )
                    pr[hh] = probs_pool.tile([128, QW], bf16, tag="pr", name="pr_t")
                    nc.scalar.activation(
                        pr[hh][:, qoff:QW], sp[:, qoff:QW],
                        mybir.ActivationFunctionType.Exp,
                        scale=0.125,
                    )
                for hh in range(2):
                    h = 2 * pair + hh
                    nc.tensor.matmul(
                        otp[hh][:, qoff:QW],
                        v_sb[kt][:, h, :],
                        pr[hh][:, qoff:QW],
                        start=(kt == 0), stop=(kt == nkt - 1),
                    )
            for hh in range(2):
                nc.vector.tensor_copy(
                    otu_sb[pair][64 * hh:64 * hh + 64, qstart:qstart + QW],
                    otp[hh][0:HD, :],
                )
                nc.vector.reciprocal(
                    rec_sb[pair][32 * hh:32 * hh + 1, qstart:qstart + QW],
                    otp[hh][HD:HD + 1, :],
                )

        ag_in = [[dram_pool.tile([128, S if p == 0 else HS], bf16,
                                 tag=f"agin{p}{h}", name=f"agin{p}{h}")
                  for h in range(2)] for p in range(2)]
        ag_out = [[dram_pool.tile([512, S if p == 0 else HS], bf16,
                                  tag=f"agout{p}{h}", name=f"agout{p}{h}")
                   for h in range(2)] for p in range(2)]
        dscr = [[dram_pool.tile([2, S], f32, tag=f"dscr{p}{h}", name=f"dscr{p}{h}")
                 for h in range(2)] for p in range(2)]

        def normalize_and_gather(pair, half, width=1):
            """Normalize token span of the pair's O^T and gather it."""
            cs = slice(half * HS, (half + width) * HS)
            w = width * HS
            eng = nc.gpsimd if pair == 0 else nc.scalar
            d = dscr[pair][half]
            eng.dma_start(out=d[0:1, 0:w], in_=rec_sb[pair][0:1, cs])
            eng.dma_start(out=d[1:2, 0:w], in_=rec_sb[pair][32:33, cs])
            for hh in range(2):
                eng.dma_start(
                    out=bc_sb[pair][64 * hh:64 * hh + 64, cs],
                    in_=d[hh:hh + 1, 0:w].to_broadcast([64, w]),
                )
            nc.vector.tensor_mul(ot_sb[pair][:, cs], otu_sb[pair][:, cs],
                                 bc_sb[pair][:, cs])
            nc.sync.dma_start(out=ag_in[pair][half][:, 0:w], in_=ot_sb[pair][:, cs])
            nc.gpsimd.collective_compute(
                "AllGather",
                mybir.AluOpType.bypass,
                replica_groups=[[0, 1, 2, 3], [4, 5, 6, 7]],
                ins=[ag_in[pair][half][:, 0:w].opt()],
                outs=[ag_out[pair][half][:, 0:w].opt()],
            )
            for r in range(4):
                nc.sync.dma_start(
                    out=otf_sb[2 * r + pair][:, cs],
                    in_=ag_out[pair][half][r * 128:(r + 1) * 128, 0:w],
                )

        def zproj(mt, ks, first, last):
            """Out-proj wave for token tile mt over contraction tiles ks."""
            ps = p1ps.tile([128, GD], f32, tag="p1", name="zps_t")
            for i, k in enumerate(ks):
                nc.tensor.matmul(
                    ps[:, 0:GD],
                    otf_sb[k][:, mt * 128:(mt + 1) * 128],
                    wout_sb[k][:],
                    start=(i == 0), stop=(i == len(ks) - 1),
                )
            return ps

        # ---- pair 0 attention interleaved with projections ----
        for qc in range(NQC):
            qkt_chunk(0, qc)
            qkt_chunk(2, qc)
            for t in range(4 * qc, 4 * qc + 4):
                v_tile(t)
            attn_qc(0, qc)
            qkt_chunk(1, qc)
            qkt_chunk(3, qc)
        normalize_and_gather(0, 0, width=2)

        # ---- pair 1 attention: gather half 0 early (hides under qc 2,3) ----
        for qc in (0, 1):
            attn_qc(1, qc)
        normalize_and_gather(1, 0)
        for qc in (2, 3):
            attn_qc(1, qc)
        normalize_and_gather(1, 1)

        # ---- out-proj pass 1: even k (pair-0 dims), backfills PE idle ----
        evens = [0, 2, 4, 6]
        odds = [1, 3, 5, 7]
        for mt in range(S // 128):
            ps = zproj(mt, evens, True, False)
            nc.vector.tensor_copy(zev_sb[mt][:], ps[:, 0:GD])

        # ---- out-proj pass 2: odd k + combine + store ----
        for i, mt in enumerate(range(S // 128)):
            ps = zproj(mt, odds, False, True)
            zrow = z_pool.tile([128, GD], f32, tag="zrow", name="zrow_t")
            nc.vector.tensor_add(zrow[:], ps[:, 0:GD], zev_sb[mt][:])
            eng = nc.sync if i % 2 == 0 else nc.scalar
            eng.dma_start(out=z_out[mt * 128:(mt + 1) * 128, :], in_=zrow[:])

    nc.compile()
    return nc


def _get_program():
    if "nc" not in _CACHE:
        _CACHE["nc"] = _build_program()
    return _CACHE["nc"]


def _make_in_maps(x, w_qkv, w_out):
    bf = ml_dtypes.bfloat16
    mneg = (np.eye(128, dtype=np.float32) * NEG).astype(bf)
    # rhs[d, q] = 1 where q < d  ->  mneg.T @ mtri adds NEG below the diagonal
    mtri = np.tril(np.ones((128, 128), dtype=np.float32), -1).astype(bf)
    in_maps = []
    for c in range(NCORES):
        b, g = c // 4, c % 4
        cs = slice(GD * g, GD * (g + 1))
        xt = np.ascontiguousarray(x[b].T).astype(bf)
        wqk = np.concatenate(
            [w_qkv[:, cs], w_qkv[:, D + GD * g:D + GD * (g + 1)]], axis=1
        ).astype(bf)
        wv = np.ascontiguousarray(w_qkv[:, 2 * D + GD * g:2 * D + GD * (g + 1)]).astype(bf)
        wo = np.ascontiguousarray(w_out[:, cs]).astype(bf)
        in_maps.append(
            {"xt": xt, "wqk": wqk, "wv": wv, "wout": wo,
             "mneg": mneg, "mtri": mtri})
    return in_maps


def kernel(x, w_qkv, b_qkv, w_out, b_out):
    from concourse.bass_utils import run_bass_kernel_spmd

    x = np.asarray(x, dtype=np.float32)
    w_qkv = np.asarray(w_qkv, dtype=np.float32)
    w_out = np.asarray(w_out, dtype=np.float32)

    nc = _get_program()
    in_maps = _make_in_maps(x, w_qkv, w_out)
    res = run_bass_kernel_spmd(nc, in_maps, list(range(NCORES))).results

    out = np.empty((B, S, D), dtype=np.float32)
    for c in range(NCORES):
        b, g = c // 4, c % 4
        out[b, :, GD * g:GD * (g + 1)] = res[c]["z"]
    return out



# revision 10
# speedup vs baseline: 1.2691x; 1.2691x over previous
"""GPT2 self-attention on 8 trn2 NeuronCores (tensor-parallel).

Sharding: core c handles batch b = c//4 and head-group g = c%4
(4 of 16 heads = 256 of 1024 dims).

Per core, qc-major (512-token query chunks), head pairs sequential:
  1. QK^T projection per chunk: [512 qk-dims, 512 tokens] = wqk^T @ x
  2. V projection per chunk:    [512 tokens, 256 v-dims]  = x @ wv
  3. Attention per (pair, chunk), keys on PSUM partitions:
       S^T[k, q] for both heads of the pair in one 2-bank PSUM tile
       exp(S/8) merged over both heads on ACT -> probs bf16
       causal diag handled by multiplying probs with a keep-mask (DVE)
       flipped AV: O[q, 65] += probs[:, qt].T @ [V | 1]  (N=65 on PE)
       per-partition reciprocal + tensor_scalar normalize -> O_norm bf16
       PE transpose [128q, 128d] -> O^T chunk
  4. Three packed AllGathers (group of 4) over token ranges
     (0:1024, 1024:1536, 1536:2048), both pairs packed per gather.
  5. Output projection per gathered token tile: z[128t, 256] via 8
     k-tiles of O^T_full against a host-sliced w_out column shard.

Host only reorders/slices/casts inputs (x^T, weight slices, bf16) and
places the 8 per-core z column-chunks into [B, S, D]. b_qkv/b_out are
zeros by the problem spec and folded out. Matmuls run bf16 with fp32
PSUM accumulation.
"""

import numpy as np
import ml_dtypes
from contextlib import ExitStack

B, S, D, H = 2, 2048, 1024, 16
HD = 64            # head dim
NCORES = 8
HPC = 4            # heads per core
GD = HPC * HD      # 256 dims per core group
QW = 512           # query-chunk width
NQC = S // QW      # 4 query chunks
NKT = S // 128     # 16 key tiles

# token ranges per gather: (start_tile, end_tile) in 128-token tiles
GATHERS = [(0, 8), (8, 12), (12, 16)]

_CACHE = {}


def _build_program():
    import concourse.tile as tile
    from concourse import bacc, mybir

    bf16 = mybir.dt.bfloat16
    f32 = mybir.dt.float32

    nc = bacc.Bacc("TRN2", target_bir_lowering=False, debug=False,
                   num_devices=NCORES)

    xt = nc.dram_tensor("xt", [D, S], bf16, kind="ExternalInput").ap()
    wqk = nc.dram_tensor("wqk", [D, 2 * GD], bf16, kind="ExternalInput").ap()
    wv = nc.dram_tensor("wv", [D, GD], bf16, kind="ExternalInput").ap()
    wout = nc.dram_tensor("wout", [D, GD], bf16, kind="ExternalInput").ap()
    keep2 = nc.dram_tensor("keep2", [128, 256], bf16, kind="ExternalInput").ap()
    ident = nc.dram_tensor("ident", [128, 128], bf16, kind="ExternalInput").ap()
    z_out = nc.dram_tensor("z", [S, GD], f32, kind="ExternalOutput").ap()

    KD = D // 128           # 8 contraction tiles over d_model

    with tile.TileContext(nc) as tc, ExitStack() as ctx:
        persist = ctx.enter_context(tc.tile_pool(name="persist", bufs=1))
        # PSUM budget: sc(2x2) + ot(2x1) + proj(2x1) = 8 banks
        sc_ps = ctx.enter_context(tc.tile_pool(name="sc_ps", bufs=2, space="PSUM"))
        ot_ps = ctx.enter_context(tc.tile_pool(name="ot_ps", bufs=2, space="PSUM"))
        pj_ps = ctx.enter_context(tc.tile_pool(name="pj_ps", bufs=2, space="PSUM"))
        on_pool = ctx.enter_context(tc.tile_pool(name="on_pool", bufs=6))
        rec_pool = ctx.enter_context(tc.tile_pool(name="rec_pool", bufs=4))
        zs_pool = ctx.enter_context(tc.tile_pool(name="zs_pool", bufs=3))
        dram_pool = ctx.enter_context(tc.tile_pool(name="dram_pool", bufs=1, space="DRAM"))

        xt_sb = [persist.tile([128, S], bf16, tag=f"xt{k}", name=f"xt{k}") for k in range(KD)]
        wqk_sb = [persist.tile([128, 2 * GD], bf16, tag=f"wqk{k}", name=f"wqk{k}") for k in range(KD)]
        wv_sb = [persist.tile([128, GD], bf16, tag=f"wv{k}", name=f"wv{k}") for k in range(KD)]
        wout_sb = [persist.tile([128, GD], bf16, tag=f"wout{k}", name=f"wout{k}") for k in range(KD)]
        keep_sb = persist.tile([128, 2, 128], bf16, tag="keep", name="keep_sb")
        ident_sb = persist.tile([128, 128], bf16, tag="ident", name="ident_sb")
        qkt_sb = [persist.tile([128, S], bf16, tag=f"qkt{m}", name=f"qkt{m}") for m in range(4)]
        v_sb = [persist.tile([128, HPC, HD + 1], bf16, tag=f"v{t}", name=f"v{t}") for t in range(NKT)]
        # probs for one whole chunk (all key tiles), double-buffered by pair
        pr_sb = [[persist.tile([128, 2, QW], bf16, tag=f"pr{pp}_{kt}",
                               name=f"pr{pp}_{kt}") for kt in range(NKT)]
                 for pp in range(2)]
        otT_sb = [persist.tile([128, S], bf16, tag=f"otT{p}", name=f"otT{p}") for p in range(2)]
        otf_sb = [persist.tile([128, S], bf16, tag=f"otf{k}", name=f"otf{k}") for k in range(KD)]

        # ---- initial loads, spread across DMA queues; first-needed cols
        # (chunk 0) land first so the QK^T projection starts early ----
        for k in range(KD):
            eng = (nc.sync, nc.scalar)[k % 2]
            eng.dma_start(out=xt_sb[k][:, 0:QW], in_=xt[k * 128:(k + 1) * 128, 0:QW])
        for k in range(KD):
            nc.gpsimd.dma_start(out=wqk_sb[k][:], in_=wqk[k * 128:(k + 1) * 128, :])
        for k in range(KD):
            eng = (nc.sync, nc.scalar)[k % 2]
            eng.dma_start(out=xt_sb[k][:, QW:S], in_=xt[k * 128:(k + 1) * 128, QW:S])
        for k in range(KD):
            nc.gpsimd.dma_start(out=wv_sb[k][:], in_=wv[k * 128:(k + 1) * 128, :])
        nc.gpsimd.dma_start(
            out=keep_sb[:], in_=keep2[:].rearrange("p (h q) -> p h q", h=2))
        nc.gpsimd.dma_start(out=ident_sb[:], in_=ident[:])
        for k in range(KD):
            nc.gpsimd.dma_start(out=wout_sb[k][:], in_=wout[k * 128:(k + 1) * 128, :])

        # ---- filler machinery: projection matmuls injected into the PE
        # stream between attention ops to hide exp latency ----
        filler_q = []  # list of thunks, each issuing ONE PE matmul (+ tail)

        def add_qkt_chunk(m, qc):
            """QK^T projection: out [128 qk-dims, 512 tokens] for chunk qc."""
            st = {}

            def step(k, st=st, m=m, qc=qc):
                if k == 0:
                    st["ps"] = pj_ps.tile([128, QW], f32, tag="pj", name="qkt_ps")
                nc.tensor.matmul(
                    st["ps"][:],
                    wqk_sb[k][:, m * 128:(m + 1) * 128],
                    xt_sb[k][:, qc * QW:(qc + 1) * QW],
                    start=(k == 0), stop=(k == KD - 1),
                )
                if k == KD - 1:
                    nc.vector.tensor_copy(
                        qkt_sb[m][:, qc * QW:(qc + 1) * QW], st["ps"][:])

            for k in range(KD):
                filler_q.append(lambda k=k: step(k))

        def add_v_tile(t):
            """V projection: out [128 tokens, 256 v-dims] for token tile t."""
            st = {}

            def step(k, st=st, t=t):
                if k == 0:
                    st["ps"] = pj_ps.tile([128, GD], f32, tag="pj", name="v_ps")
                nc.tensor.matmul(
                    st["ps"][:, 0:GD],
                    xt_sb[k][:, t * 128:(t + 1) * 128],
                    wv_sb[k][:],
                    start=(k == 0), stop=(k == KD - 1),
                )
                if k == KD - 1:
                    nc.vector.tensor_copy(
                        v_sb[t][:, :, 0:HD],
                        st["ps"][:, 0:GD].rearrange("p (h d) -> p h d", h=HPC),
                    )
                    nc.vector.memset(v_sb[t][:, :, HD:HD + 1], 1.0)

            for k in range(KD):
                filler_q.append(lambda k=k: step(k))

        def add_zproj(mt):
            """Out-proj for token tile mt: z[128t, 256] over 8 k-tiles."""
            st = {}

            def step(k, st=st, mt=mt):
                if k == 0:
                    st["ps"] = pj_ps.tile([128, GD], f32, tag="pj", name="z_ps")
                nc.tensor.matmul(
                    st["ps"][:, 0:GD],
                    otf_sb[k][:, mt * 128:(mt + 1) * 128],
                    wout_sb[k][:],
                    start=(k == 0), stop=(k == KD - 1),
                )
                if k == KD - 1:
                    zrow = zs_pool.tile([128, GD], f32, tag="zrow", name="zrow_t")
                    nc.vector.tensor_copy(zrow[:], st["ps"][:, 0:GD])
                    nc.gpsimd.dma_start(
                        out=z_out[mt * 128:(mt + 1) * 128, :], in_=zrow[:])

            for k in range(KD):
                filler_q.append(lambda k=k: step(k))

        def fill(n):
            for _ in range(n):
                if filler_q:
                    filler_q.pop(0)()

        def drain_fillers():
            while filler_q:
                filler_q.pop(0)()

        # ---- attention for (pair, qc) ----
        def attn(pair, qc):
            qstart = qc * QW
            nkt = 4 * (qc + 1)
            prs = pr_sb[pair]
            # scores + exp for every key tile of the chunk
            for kt in range(nkt):
                j = kt - 4 * qc           # diag sub-tile index if >= 0
                qoff = max(0, 128 * j)
                sp = sc_ps.tile([128, 2, QW], f32, tag="sc", name="sc_t")
                for hh in range(2):
                    base = 64 * hh
                    nc.tensor.matmul(
                        sp[:, hh, qoff:QW],
                        qkt_sb[2 + pair][base:base + 64, kt * 128:(kt + 1) * 128],
                        qkt_sb[pair][base:base + 64, qstart + qoff:qstart + QW],
                        start=True, stop=True,
                    )
                nc.scalar.activation(
                    prs[kt][:, :, qoff:QW], sp[:, :, qoff:QW],
                    mybir.ActivationFunctionType.Exp,
                    scale=0.125,
                )
                if j >= 0:
                    # zero the strictly-upper (key > query) part of the
                    # diagonal 128x128 block, both heads at once
                    nc.vector.tensor_mul(
                        prs[kt][:, :, qoff:qoff + 128],
                        prs[kt][:, :, qoff:qoff + 128],
                        keep_sb[:],
                    )
                fill(3)
            # AV, qt-outer so each PSUM region accumulates in one
            # consecutive group (interleaved groups in a bank are broken
            # on HW); normalize + transpose each qt as soon as it stops
            otp = [ot_ps.tile([128, 4, HD + 1], f32, tag="ot", name="otp_t")
                   for _ in range(2)]
            rec = rec_pool.tile([128, 2, 4], f32, tag="rec", name="rec_t")
            for l in range(4):
                qt = 4 * qc + l
                for hh in range(2):
                    h = 2 * pair + hh
                    for kt in range(qt + 1):
                        nc.tensor.matmul(
                            otp[hh][:, l, :],
                            prs[kt][:, hh, l * 128:(l + 1) * 128],
                            v_sb[kt][:, h, :],
                            start=(kt == 0), stop=(kt == qt),
                        )
                    nc.vector.reciprocal(rec[:, hh, l:l + 1],
                                         otp[hh][:, l, HD:HD + 1])
                onorm = on_pool.tile([128, 128], bf16, tag="on", name="on_t")
                for hh in range(2):
                    nc.vector.tensor_scalar_mul(
                        onorm[:, 64 * hh:64 * hh + 64],
                        otp[hh][:, l, 0:HD],
                        rec[:, hh, l:l + 1],
                    )
                tp = pj_ps.tile([128, 128], bf16, tag="pj", name="tp_t")
                nc.tensor.transpose(tp[:], onorm[:], ident_sb[:])
                nc.vector.tensor_copy(
                    otT_sb[pair][:, qstart + 128 * l:qstart + 128 * (l + 1)],
                    tp[:],
                )
                fill(2)

        # ---- gathers ----
        ag_in = [dram_pool.tile([128, 2, 128 * (t1 - t0)], bf16,
                                tag=f"agin{gi}", name=f"agin{gi}")
                 for gi, (t0, t1) in enumerate(GATHERS)]
        ag_out = [dram_pool.tile([512, 2, 128 * (t1 - t0)], bf16,
                                 tag=f"agout{gi}", name=f"agout{gi}")
                  for gi, (t0, t1) in enumerate(GATHERS)]

        def gather(gi):
            t0, t1 = GATHERS[gi]
            w = 128 * (t1 - t0)
            for pair in range(2):
                nc.sync.dma_start(
                    out=ag_in[gi][:, pair, :],
                    in_=otT_sb[pair][:, 128 * t0:128 * t1],
                )
            nc.gpsimd.collective_compute(
                "AllGather",
                mybir.AluOpType.bypass,
                replica_groups=[[0, 1, 2, 3], [4, 5, 6, 7]],
                ins=[ag_in[gi][:].opt()],
                outs=[ag_out[gi][:].opt()],
            )
            for r in range(4):
                for pair in range(2):
                    eng = nc.sync if pair == 0 else nc.gpsimd
                    eng.dma_start(
                        out=otf_sb[2 * r + pair][:, 128 * t0:128 * t1],
                        in_=ag_out[gi][128 * r:128 * (r + 1), pair, :],
                    )

        # ---- main schedule ----
        # qc0 projections run directly (DMA-paced), rest become fillers
        add_qkt_chunk(0, 0); add_qkt_chunk(1, 0)
        add_qkt_chunk(2, 0); add_qkt_chunk(3, 0)
        for t in range(4):
            add_v_tile(t)
        drain_fillers()

        for qc in range(NQC):
            # stage fillers: next chunk's projections / unlocked out-proj
            if qc + 1 < NQC:
                for m in range(4):
                    add_qkt_chunk(m, qc + 1)
                for t in range(4 * qc + 4, 4 * qc + 8):
                    add_v_tile(t)
            else:
                for mt in range(12):
                    add_zproj(mt)       # gathers 0/1 token tiles
            attn(0, qc)
            attn(1, qc)
            if qc == 1:
                gather(0)
            elif qc == 2:
                gather(1)
            elif qc == 3:
                drain_fillers()
                gather(2)
                for mt in range(12, 16):
                    add_zproj(mt)
                drain_fillers()

    nc.compile()
    return nc


def _get_program():
    if "nc" not in _CACHE:
        _CACHE["nc"] = _build_program()
    return _CACHE["nc"]


def _make_in_maps(x, w_qkv, w_out):
    bf = ml_dtypes.bfloat16
    keep = np.triu(np.ones((128, 128), dtype=np.float32))
    keep2 = np.concatenate([keep, keep], axis=1).astype(bf)
    ident = np.eye(128, dtype=np.float32).astype(bf)
    in_maps = []
    for c in range(NCORES):
        b, g = c // 4, c % 4
        cs = slice(GD * g, GD * (g + 1))
        xt = np.ascontiguousarray(x[b].T).astype(bf)
        wqk = np.concatenate(
            [w_qkv[:, cs], w_qkv[:, D + GD * g:D + GD * (g + 1)]], axis=1
        ).astype(bf)
        wv = np.ascontiguousarray(w_qkv[:, 2 * D + GD * g:2 * D + GD * (g + 1)]).astype(bf)
        wo = np.ascontiguousarray(w_out[:, cs]).astype(bf)
        in_maps.append(
            {"xt": xt, "wqk": wqk, "wv": wv, "wout": wo,
             "keep2": keep2, "ident": ident})
    return in_maps


def kernel(x, w_qkv, b_qkv, w_out, b_out):
    from concourse.bass_utils import run_bass_kernel_spmd

    x = np.asarray(x, dtype=np.float32)
    w_qkv = np.asarray(w_qkv, dtype=np.float32)
    w_out = np.asarray(w_out, dtype=np.float32)

    nc = _get_program()
    in_maps = _make_in_maps(x, w_qkv, w_out)
    res = run_bass_kernel_spmd(nc, in_maps, list(range(NCORES))).results

    out = np.empty((B, S, D), dtype=np.float32)
    for c in range(NCORES):
        b, g = c // 4, c % 4
        out[b, :, GD * g:GD * (g + 1)] = res[c]["z"]
    return out
